# revision 12
# baseline (speedup 1.0000x reference)
"""GRU-cell-variant kernel for Trainium2, data-parallel over batch on 8 cores.

Reference (per batch row b, hidden size H=1024):
    gates = sigmoid(x @ W_ih + b_ih + h @ W_hh + b_hh)   # [B, 2H]
    z, r  = gates[:, :H], gates[:, H:]
    cand  = tanh(x @ W_c + b_c + r * (h @ W_hc + b_hc))
    out   = (1 - z) * h + z * cand

Design (v4):
  - 8-way batch shard (1024 rows/core), weights replicated. No collectives.
  - Everything on-chip is computed TRANSPOSED: out.T[o, b].
  - Matmul precision (numpy sim matches HW to ~1e-6; budget 2e-2, this
    config sims 1.936e-2): z/r gates + h@W_hc fp8e4 DoubleRow (x128
    pre-scale, 1/128 folded into ACT scale); x@W_c k-tiles 0-5 fp8e4 DR,
    k-tiles 6-7 full-rate e3m4 x fp16.  25 matmuls per 512-col block.
  - Blend: out = h + z*(cand - h): 5 wide fp16 vector ops.
  - DMA reality (measured): the 3 DMA-capable queues (gpsimd/sync/
    scalar) each run their transfers serially with ~1.2us fixed
    overhead per DMA; issue instructions cost ~0.7us of queue time.
    So: few LARGE DMAs, each queue's stream in consumption order.
    Weights alone on gpsimd (one fused fp8 "wall" DMA per j);
    b0 acts on sync, b1 acts + consts on scalar; steady-state
    wc16+h16 prefetch on scalar; outputs on sync.
  - j0 computes ALL x-side matmuls first (z/r x-pairs + xc-DR for both
    batch halves), then the h-side, so h8 gets ~7us more stream time.
    All 8 PSUM banks hold j0's accumulation groups concurrently.
  - 4 dummy warm-up matmuls on a memset scratch raise the PE out of its
    low-power state before real data lands (full clock needs ~3us of
    continuous PE activity).
  - Tail: single-piece 512-wide blend everywhere (the Vector queue is
    the serial tail bottleneck; splitting only added op overhead).
"""

import numpy as np
import ml_dtypes

import concourse.bass as bass
import concourse.mybir as mybir
import concourse.tile as tile
from concourse import bacc
from concourse.bass_utils import run_bass_kernel_spmd

N_CORES = 8
B = 8192
H = 1024
BL = B // N_CORES  # batch rows per core
P = 128
KC = H // P  # 8 contraction chunks of 128 per 1024-wide operand
NJ = H // P  # 8 hidden-dim tiles
NB = BL // 512  # 2 moving halves of 512 batch columns
XCD = 3  # x@W_c fp8e4 DoubleRow k-pairs (of 4); rest e3m4 full-rate
NE3 = KC - 2 * XCD  # e3m4 k-tiles for x@W_c
WALL = 4096 + 1792  # fp8 weight bytes per j: Wg z|r (4096) + Whc|Wc8 (1792)

WS = 128.0  # host-side weight pre-scale (power of two, exact)

F8 = mybir.dt.float8e4
E3 = mybir.dt.float8e3
F16 = mybir.dt.float16
F32 = mybir.dt.float32
AF = mybir.ActivationFunctionType
ALU = mybir.AluOpType
DR = mybir.MatmulPerfMode.DoubleRow

_CACHE = {}


def _build_program():
    nc = bacc.Bacc(
        "TRN2",
        target_bir_lowering=False,
        debug=False,
        enable_asserts=False,
        num_devices=N_CORES,
    )

    # DRAM inputs, packed on the host into SBUF-friendly layouts.
    # x8/h8:  [p, hb*KC*512 + kc*512 + c] = x[hb*512+c, kc*128+p]  (fp8e4)
    # xe3:    [p, hb*NE3*512 + t*512 + c] = x[hb*512+c, (6+t)*128+p]
    # h16:    [p, j*BL + b] fp16 (residual path)
    # Wall:   [p, j*5888 + g*128 + jj]; g<16: z k-tile g; g<32: r k-tile
    #         g-16; g<40: whc k-tile g-32; g<46: wc8 k-tile g-40. x128 fp8e4
    # Wc16:   [p, j*256 + t*128 + jj] = 128*W_c[(6+t)*128+p, j*128+jj]
    # bg:     [p, t] = (b_ih+b_hh)[t*128+p] unscaled; bc unscaled; bhc x128
    x8 = nc.dram_tensor("x8", [P, KC * BL], F8, kind="ExternalInput").ap()
    h8 = nc.dram_tensor("h8", [P, KC * BL], F8, kind="ExternalInput").ap()
    xe3 = nc.dram_tensor("xe3", [P, NE3 * BL], E3, kind="ExternalInput").ap()
    h16 = nc.dram_tensor("h16", [P, NJ * BL], F16, kind="ExternalInput").ap()
    Wall = nc.dram_tensor("Wall", [P, NJ * WALL], F8, kind="ExternalInput").ap()
    Wc16 = nc.dram_tensor("Wc16", [P, NJ * NE3 * P], F16, kind="ExternalInput").ap()
    bg = nc.dram_tensor("bg", [P, 16], F32, kind="ExternalInput").ap()
    bc = nc.dram_tensor("bc", [P, NJ], F32, kind="ExternalInput").ap()
    bhc = nc.dram_tensor("bhc", [P, NJ], F32, kind="ExternalInput").ap()
    outT = nc.dram_tensor("outT", [P, NJ * BL], F16, kind="ExternalOutput").ap()

    inv = 1.0 / WS

    with tile.TileContext(nc) as tc:
        with (
            tc.tile_pool(name="const", bufs=1) as cpool,
            tc.tile_pool(name="wg", bufs=3) as wgpool,
            tc.tile_pool(name="psum", bufs=8, space="PSUM") as ppool,
            tc.tile_pool(name="gates", bufs=6) as gpool,
            tc.tile_pool(name="work", bufs=8) as wpool,
        ):
            bg_sb = cpool.tile([P, 16], F32, tag="bg")
            bc_sb = cpool.tile([P, NJ], F32, tag="bc")
            bhc_sb = cpool.tile([P, NJ], F32, tag="bhc")
            warm = cpool.tile([P, 1280], F8, tag="warm")

            x8_sb = cpool.tile([P, KC * BL], F8, tag="x8")
            h8_sb = cpool.tile([P, KC * BL], F8, tag="h8")
            xe3_sb = cpool.tile([P, NE3 * BL], E3, tag="xe3")
            h16_sb = cpool.tile([P, NJ * BL], F16, tag="h16")

            # 4D views [p, hb, kc, c]: batch-half major so half-loads are
            # single contiguous runs per partition.
            xs8 = x8_sb[:].rearrange("p (hb kc c) -> p hb kc c", hb=NB, kc=KC)
            xd8 = x8.rearrange("p (hb k) -> p hb k", hb=NB)
            hs8 = h8_sb[:].rearrange("p (hb kc c) -> p hb kc c", hb=NB, kc=KC)
            hd8 = h8.rearrange("p (hb k) -> p hb k", hb=NB)
            xs8f = x8_sb[:].rearrange("p (hb k) -> p hb k", hb=NB)
            hs8f = h8_sb[:].rearrange("p (hb k) -> p hb k", hb=NB)
            xs3 = xe3_sb[:].rearrange("p (hb t c) -> p hb t c", hb=NB, t=NE3)

            def gate_pairs(psum, w3, goff, hb, qs, start, stop, o0=0, w=512):
                # pairs 0-3 read x8, pairs 4-7 read h8 (16 k-tiles over [x;h])
                for i, q in enumerate(qs):
                    src = xs8 if q < 4 else hs8
                    kk = (q % 4) * 2
                    g = goff + 2 * q
                    nc.tensor.matmul(
                        psum,
                        lhsT=w3[:, g : g + 2, :],
                        rhs=src[:, hb, kk : kk + 2, o0 : o0 + w],
                        start=(start and i == 0),
                        stop=(stop and i == len(qs) - 1),
                        perf_mode=DR,
                    )

            def hc_dr(psum, w3, hb, o0=0, w=512):
                for q in range(4):
                    nc.tensor.matmul(
                        psum,
                        lhsT=w3[:, 32 + 2 * q : 32 + 2 * q + 2, :],
                        rhs=hs8[:, hb, 2 * q : 2 * q + 2, o0 : o0 + w],
                        start=(q == 0),
                        stop=(q == 3),
                        perf_mode=DR,
                    )

            def xc_dr(psum, w3, hb, start=True, o0=0, w=512):
                for q in range(XCD):
                    nc.tensor.matmul(
                        psum,
                        lhsT=w3[:, 40 + 2 * q : 40 + 2 * q + 2, :],
                        rhs=xs8[:, hb, 2 * q : 2 * q + 2, o0 : o0 + w],
                        start=(start and q == 0),
                        stop=False,
                        perf_mode=DR,
                    )

            def xc_e3(psum, wc16_w, hb, o0=0, w=512):
                for t in range(NE3):
                    nc.tensor.matmul(
                        psum,
                        lhsT=wc16_w[:, t * P : (t + 1) * P],
                        rhs=xs3[:, hb, t, o0 : o0 + w],
                        start=False,
                        stop=(t == NE3 - 1),
                    )

            def alloc_w():
                wall_w = wgpool.tile([P, WALL], F8, tag="wall")
                wc16_w = wgpool.tile([P, NE3 * P], F16, tag="wc16")
                return (
                    wall_w, wc16_w,
                    wall_w[:].rearrange("p (g m) -> p g m", g=46),
                )

            def load_w(tiles, j):
                wall_w, wc16_w = tiles[:2]
                nc.gpsimd.dma_start(wall_w[:], Wall[:, j * WALL : (j + 1) * WALL])
                nc.scalar.dma_start(
                    wc16_w[:], Wc16[:, j * NE3 * P : (j + 1) * NE3 * P]
                )
                nc.scalar.dma_start(
                    h16_sb[:, j * BL : (j + 1) * BL], h16[:, j * BL : (j + 1) * BL]
                )

            def sig(psum, j, goff):
                g_sb = gpool.tile([P, 512], F16, tag="g")
                nc.scalar.activation(
                    g_sb[:], psum[:], AF.Sigmoid,
                    bias=bg_sb[:, goff + j : goff + j + 1], scale=inv,
                )
                return g_sb

            def blend(j, hoff, ph, px, r_sb, z_sb, w=512, pieces=1):
                # cand = tanh((px + r*(ph + bhc))/WS + bc)
                # out  = h + z*(cand - h)
                rh = wpool.tile([P, 512], F16, tag="rh")
                s = wpool.tile([P, 512], F32, tag="s")
                cand = wpool.tile([P, 512], F16, tag="cand")
                d = wpool.tile([P, 512], F16, tag="d")
                m = wpool.tile([P, 512], F16, tag="m")
                o_sb = wpool.tile([P, 512], F16, tag="o")
                pw = w // pieces
                for pc in range(pieces):
                    sl = slice(pc * pw, (pc + 1) * pw)
                    hsl = h16_sb[:, hoff + pc * pw : hoff + (pc + 1) * pw]
                    nc.vector.scalar_tensor_tensor(
                        rh[:, sl], ph[:, sl], bhc_sb[:, j : j + 1], r_sb[:, sl],
                        ALU.add, ALU.mult,
                    )
                    nc.vector.tensor_add(s[:, sl], px[:, sl], rh[:, sl])
                    nc.scalar.activation(
                        cand[:, sl], s[:, sl], AF.Tanh,
                        bias=bc_sb[:, j : j + 1], scale=inv,
                    )
                    nc.vector.tensor_sub(d[:, sl], cand[:, sl], hsl)
                    nc.vector.tensor_mul(m[:, sl], z_sb[:, sl], d[:, sl])
                    nc.vector.tensor_add(o_sb[:, sl], m[:, sl], hsl)
                    nc.sync.dma_start(
                        outT[:, hoff + pc * pw : hoff + (pc + 1) * pw], o_sb[:, sl]
                    )

            # ---------------- j = 0: cold start ----------------
            cur = alloc_w()
            nxt = alloc_w()
            wall0, wc160, w03 = cur

            # DMA reality: each queue's transfers are serial with ~1.2us of
            # fixed per-DMA overhead and ~3us queue startup, so use FEW,
            # LARGE DMAs, each queue's stream in consumption order, three
            # queues overlapping.
            # sync: b0 activations, then j0 residual; outputs later.
            nc.sync.dma_start(xs8f[:, 0, :], xd8[:, 0, :])
            nc.sync.dma_start(hs8f[:, 0, :], hd8[:, 0, :])
            nc.sync.dma_start(h16_sb[:, 0:BL], h16[:, 0:BL])
            # scalar: j0 weights in consumption order (z pair0 split off so
            # the first LDWEIGHTS starts early), then the small consts.
            nc.scalar.dma_start(wall0[:, 0:256], Wall[:, 0:256])
            nc.scalar.dma_start(wall0[:, 256:2048], Wall[:, 256:2048])
            nc.scalar.dma_start(wall0[:, 2048:4096], Wall[:, 2048:4096])
            nc.scalar.dma_start(bg_sb[:], bg[:])
            nc.scalar.dma_start(wall0[:, 4096:WALL], Wall[:, 4096:WALL])
            nc.scalar.dma_start(wc160[:], Wc16[:, 0 : NE3 * P])
            nc.scalar.dma_start(xe3_sb[:], xe3[:])
            nc.scalar.dma_start(bhc_sb[:], bhc[:])
            nc.scalar.dma_start(bc_sb[:], bc[:])
            # gpsimd: b1 activations, then j1's weights; steady walls later.
            nc.gpsimd.dma_start(xs8f[:, 1, :], xd8[:, 1, :])
            nc.gpsimd.dma_start(hs8f[:, 1, :], hd8[:, 1, :])
            nc.gpsimd.dma_start(nxt[0][:], Wall[:, WALL : 2 * WALL])
            nc.scalar.dma_start(nxt[1][:], Wc16[:, NE3 * P : 2 * NE3 * P])
            nc.scalar.dma_start(h16_sb[:, BL : 2 * BL], h16[:, BL : 2 * BL])

            # PE warm-up: dummy DR matmuls on memset scratch keep the PE
            # continuously busy until real data arrives, so it reaches full
            # clock (needs ~3us of uninterrupted activity) before real work.
            nc.vector.memset(warm[:], 0)
            wv3 = warm[:, 0:256].rearrange("p (g m) -> p g m", g=2)
            wrv = warm[:, 256:768].rearrange("p (k c) -> p k c", k=2)
            pwm = ppool.tile([P, 512], F32, tag="ps")
            for i in range(14):
                nc.tensor.matmul(
                    pwm[:, 0:256], lhsT=wv3[:, 0:2, :], rhs=wrv[:],
                    start=True, stop=True, perf_mode=DR,
                )

            # j0: all x-side work first (h8 streams meanwhile), h-side after,
            # each phase ordered z-before-r to match weight arrival.
            pz0 = ppool.tile([P, 512], F32, tag="ps")
            pr0 = ppool.tile([P, 512], F32, tag="ps")
            pz1 = ppool.tile([P, 512], F32, tag="ps")
            pr1 = ppool.tile([P, 512], F32, tag="ps")
            px0 = ppool.tile([P, 512], F32, tag="ps")
            px1 = ppool.tile([P, 512], F32, tag="ps")
            gate_pairs(pz0[:], w03, 0, 0, range(4), True, False)
            gate_pairs(pr0[:], w03, 16, 0, range(4), True, False)
            gate_pairs(pz1[:], w03, 0, 1, range(4), True, False)
            gate_pairs(pr1[:], w03, 16, 1, range(4), True, False)
            xc_dr(px0[:], w03, 0)
            xc_dr(px1[:], w03, 1)
            # h-side, b0 then b1; close each group as late data allows.
            ph0 = ppool.tile([P, 512], F32, tag="ps")
            gate_pairs(pz0[:], w03, 0, 0, (4, 5, 6, 7), False, True)
            gate_pairs(pr0[:], w03, 16, 0, (4, 5, 6, 7), False, True)
            z0 = sig(pz0, 0, 0)
            r0 = sig(pr0, 0, NJ)
            hc_dr(ph0[:], w03, 0)
            xc_e3(px0[:], wc160, 0)
            blend(0, 0, ph0, px0, r0[:], z0[:])
            ph1 = ppool.tile([P, 512], F32, tag="ps")
            gate_pairs(pz1[:], w03, 0, 1, (4, 5, 6, 7), False, True)
            gate_pairs(pr1[:], w03, 16, 1, (4, 5, 6, 7), False, True)
            z1 = sig(pz1, 0, 0)
            r1 = sig(pr1, 0, NJ)
            hc_dr(ph1[:], w03, 1)
            xc_e3(px1[:], wc160, 1)
            blend(0, 512, ph1, px1, r1[:], z1[:])

            cur = nxt

            # ---------------- j = 1..7: steady state ----------------
            for j in range(1, NJ):
                if j + 1 < NJ:
                    nxt = alloc_w()
                    load_w(nxt, j + 1)
                wall_w, wc16_w, w3 = cur

                for b in range(NB):
                    hoff = j * BL + b * 512
                    last = j == NJ - 1 and b == NB - 1

                    pz = ppool.tile([P, 512], F32, tag="ps")
                    gate_pairs(pz[:], w3, 0, b, range(8), True, True)
                    z_sb = sig(pz, j, 0)
                    pr = ppool.tile([P, 512], F32, tag="ps")
                    gate_pairs(pr[:], w3, 16, b, range(8), True, True)
                    r_sb = sig(pr, j, NJ)

                    ph = ppool.tile([P, 512], F32, tag="ps")
                    hc_dr(ph[:], w3, b)
                    px = ppool.tile([P, 512], F32, tag="ps")
                    xc_dr(px[:], w3, b)
                    xc_e3(px[:], wc16_w, b)
                    blend(j, hoff, ph, px, r_sb[:], z_sb[:])

                cur = nxt

    nc.compile()
    return nc


def _pack_weights(W_ih, b_ih, W_hh, b_hh, W_c, b_c, W_hc, b_hc):
    f8 = ml_dtypes.float8_e4m3
    Wg_full = np.concatenate([W_ih, W_hh], axis=0)  # [2H, 2H] = [k, o]
    T = (Wg_full * WS).reshape(16, P, 16, P).transpose(1, 2, 0, 3)  # [p,ot,kt,o]
    Thc = (W_hc * WS).reshape(KC, P, NJ, P).transpose(1, 2, 0, 3)  # [p,j,kt,o]
    Tc8 = (W_c[: 2 * XCD * P] * WS).reshape(2 * XCD, P, NJ, P).transpose(1, 2, 0, 3)
    WallH = np.ascontiguousarray(
        np.concatenate(
            [
                np.stack([T[:, j] for j in range(NJ)], axis=1),  # z [p,j,16,P]
                np.stack([T[:, NJ + j] for j in range(NJ)], axis=1),  # r
                Thc,
                Tc8,
            ],
            axis=2,
        ).reshape(P, NJ * WALL)
    ).astype(f8)
    Wc16H = np.ascontiguousarray(
        (W_c[2 * XCD * P :] * WS)
        .reshape(NE3, P, NJ, P)
        .transpose(1, 2, 0, 3)
        .reshape(P, NJ * NE3 * P)
    ).astype(np.float16)
    bgH = np.ascontiguousarray((b_ih + b_hh).reshape(16, P).T).astype(np.float32)
    bcH = np.ascontiguousarray(b_c.reshape(NJ, P).T).astype(np.float32)
    bhcH = np.ascontiguousarray((b_hc * WS).reshape(NJ, P).T).astype(np.float32)
    return WallH, Wc16H, bgH, bcH, bhcH


def _pack_acts_hb(a, dtype, k0=0, k1=KC):
    # [BL, H] -> [p, hb*(k1-k0)*512 + kc*512 + c] with a[hb*512+c, kc*128+p]
    return np.ascontiguousarray(
        a.T.reshape(KC, P, NB, 512)[k0:k1]
        .transpose(1, 2, 0, 3)
        .reshape(P, NB * (k1 - k0) * 512)
    ).astype(dtype)


def _pack_h16(a):
    # [BL, H] -> [p, j*BL + b] with a[b, j*128+p]
    return np.ascontiguousarray(
        a.T.reshape(NJ, P, BL).transpose(1, 0, 2).reshape(P, NJ * BL)
    ).astype(np.float16)


def _make_in_maps(input, hx, W_ih, b_ih, W_hh, b_hh, W_c, b_c, W_hc, b_hc):
    input = np.asarray(input, np.float32)
    hx = np.asarray(hx, np.float32)
    WallH, Wc16H, bgH, bcH, bhcH = _pack_weights(
        np.asarray(W_ih, np.float32), np.asarray(b_ih, np.float32),
        np.asarray(W_hh, np.float32), np.asarray(b_hh, np.float32),
        np.asarray(W_c, np.float32), np.asarray(b_c, np.float32),
        np.asarray(W_hc, np.float32), np.asarray(b_hc, np.float32),
    )
    f8 = ml_dtypes.float8_e4m3
    e3 = ml_dtypes.float8_e3m4
    in_maps = []
    for i in range(N_CORES):
        xs = input[i * BL : (i + 1) * BL]
        hs = hx[i * BL : (i + 1) * BL]
        in_maps.append(
            {
                "x8": _pack_acts_hb(xs, f8),
                "h8": _pack_acts_hb(hs, f8),
                "xe3": _pack_acts_hb(xs, e3, k0=2 * XCD),
                "h16": _pack_h16(hs),
                "Wall": WallH,
                "Wc16": Wc16H,
                "bg": bgH,
                "bc": bcH,
                "bhc": bhcH,
            }
        )
    return in_maps


def kernel(input, hx, W_ih, b_ih, W_hh, b_hh, W_c, b_c, W_hc, b_hc):
    if "nc" not in _CACHE:
        _CACHE["nc"] = _build_program()
    nc = _CACHE["nc"]

    in_maps = _make_in_maps(
        input, hx, W_ih, b_ih, W_hh, b_hh, W_c, b_c, W_hc, b_hc
    )

    res = run_bass_kernel_spmd(nc, in_maps, core_ids=list(range(N_CORES)))
    out = np.empty((B, H), np.float32)
    for i, r in enumerate(res.results):
        o = np.asarray(r["outT"], np.float32).reshape(P, NJ, BL).transpose(2, 1, 0).reshape(BL, H)
        out[i * BL : (i + 1) * BL] = o
    return out


# revision 13
# speedup vs baseline: 1.0368x; 1.0368x over previous
"""GRU-cell-variant kernel for Trainium2, data-parallel over batch on 8 cores.

Reference (per batch row b, hidden size H=1024):
    gates = sigmoid(x @ W_ih + b_ih + h @ W_hh + b_hh)   # [B, 2H]
    z, r  = gates[:, :H], gates[:, H:]
    cand  = tanh(x @ W_c + b_c + r * (h @ W_hc + b_hc))
    out   = (1 - z) * h + z * cand

Design (v2):
  - 8-way batch shard (1024 rows/core), weights replicated. No collectives.
  - Everything on-chip is computed TRANSPOSED: out.T[o, b].
  - Matmul precision (numpy sim of quantization matches HW to ~1e-6;
    rel-err budget 2e-2, this config sims at 1.936e-2):
      * z-gate, r-gate, h@W_hc: fp8e4 double-pumped (DoubleRow, 2
        contraction k-tiles per pass = 2x PE rate), weights pre-scaled
        x128; the 1/128 folds into ACT activation scales.
      * x@W_c: k-tiles 0-5 fp8e4 DR off the resident x8; k-tiles 6-7
        full-rate e3m4 moving x, fp16 x128 stationary W_c.
    25 matmul instructions per 512-col block (vs 27 in v1).
  - Blend rewritten as out = h + z*(cand - h): 5 wide fp16 vector ops
    (rh STT, s add, d sub, m mul, o add) instead of 8 half-wide fp32 +
    2 gpsimd ops. Vector busy/block ~2.8us vs Tensor ~5.4us.
  - ALL prefetch DMAs (weights, activations, biases) issue from the
    GpSimd/Pool queue in consumption order; output DMAs ride the Sync
    queue (they carry data deps and must not block prefetch issues).
    The Scalar queue carries only the 3 activations per block so the
    blend chain never queues behind DMA issues.
  - Weight groups packed host-side so each j-tile needs 4 DMAs total:
    Wg [z|r] 4KB/line fp8, W8 [whc|wc8] 1.75KB/line fp8, Wc16 fp16,
    h16 residual stream.
  - fp32 PSUM accumulation; h-residual fp16; out fp16.
  - Tail: final block's hc/xc + blend run as 2x256-wide halves so the
    post-matmul chain is short.
"""

import numpy as np
import ml_dtypes

import concourse.bass as bass
import concourse.mybir as mybir
import concourse.tile as tile
from concourse import bacc
from concourse.bass_utils import run_bass_kernel_spmd

N_CORES = 8
B = 8192
H = 1024
BL = B // N_CORES  # batch rows per core
P = 128
KC = H // P  # 8 contraction chunks of 128 per 1024-wide operand
NJ = H // P  # 8 hidden-dim tiles
NB = BL // 512  # 2 moving halves of 512 batch columns
XCD = 3  # x@W_c fp8e4 DoubleRow k-pairs (of 4); rest e3m4 full-rate
NE3 = KC - 2 * XCD  # e3m4 k-tiles for x@W_c

WS = 128.0  # host-side weight pre-scale (power of two, exact)

F8 = mybir.dt.float8e4
E3 = mybir.dt.float8e3
F16 = mybir.dt.float16
F32 = mybir.dt.float32
AF = mybir.ActivationFunctionType
ALU = mybir.AluOpType
DR = mybir.MatmulPerfMode.DoubleRow

_CACHE = {}


def _build_program():
    nc = bacc.Bacc(
        "TRN2",
        target_bir_lowering=False,
        debug=False,
        enable_asserts=False,
        num_devices=N_CORES,
    )

    # DRAM inputs, packed on the host into SBUF-friendly layouts.
    # x8/h8:  [p, kc*BL + b]        = x[b, kc*128 + p]             (fp8e4)
    # xe3:    [p, t*BL + b]         = x[b, (6+t)*128 + p], t<2     (fp8e3)
    # h16:    [p, j*BL + b]         fp16 (residual path)
    # Wg:     [p, j*4096 + g*128 + jj], g<16: z k-tile g; g>=16: r k-tile
    #         g-16; value = 128*Wg_full[k, o]                      (fp8e4)
    # W8:     [p, j*1792 + g*128 + jj], g<8: whc k-tile; g>=8: wc8 k-tile
    #         (x@W_c k-tiles 0-5), x128                            (fp8e4)
    # Wc16:   [p, j*256 + t*128 + jj] = 128*W_c[(6+t)*128+p, j*128+jj]
    # bg:     [p, t] = (b_ih+b_hh)[t*128+p] unscaled; bc unscaled; bhc x128
    x8 = nc.dram_tensor("x8", [P, KC * BL], F8, kind="ExternalInput").ap()
    h8 = nc.dram_tensor("h8", [P, KC * BL], F8, kind="ExternalInput").ap()
    xe3 = nc.dram_tensor("xe3", [P, NE3 * BL], E3, kind="ExternalInput").ap()
    h16 = nc.dram_tensor("h16", [P, NJ * BL], F16, kind="ExternalInput").ap()
    Wg = nc.dram_tensor("Wg", [P, NJ * 4096], F8, kind="ExternalInput").ap()
    W8 = nc.dram_tensor("W8", [P, NJ * 1792], F8, kind="ExternalInput").ap()
    Wc16 = nc.dram_tensor("Wc16", [P, NJ * NE3 * P], F16, kind="ExternalInput").ap()
    bg = nc.dram_tensor("bg", [P, 16], F32, kind="ExternalInput").ap()
    bc = nc.dram_tensor("bc", [P, NJ], F32, kind="ExternalInput").ap()
    bhc = nc.dram_tensor("bhc", [P, NJ], F32, kind="ExternalInput").ap()
    outT = nc.dram_tensor("outT", [P, NJ * BL], F16, kind="ExternalOutput").ap()

    inv = 1.0 / WS

    with tile.TileContext(nc) as tc:
        with (
            tc.tile_pool(name="const", bufs=1) as cpool,
            tc.tile_pool(name="wg", bufs=3) as wgpool,
            tc.tile_pool(name="psum", bufs=8, space="PSUM") as ppool,
            tc.tile_pool(name="gates", bufs=6) as gpool,
            tc.tile_pool(name="work", bufs=8) as wpool,
        ):
            bg_sb = cpool.tile([P, 16], F32, tag="bg")
            bc_sb = cpool.tile([P, NJ], F32, tag="bc")
            bhc_sb = cpool.tile([P, NJ], F32, tag="bhc")

            x8_sb = cpool.tile([P, KC * BL], F8, tag="x8")
            h8_sb = cpool.tile([P, KC * BL], F8, tag="h8")
            xe3_sb = cpool.tile([P, NE3 * BL], E3, tag="xe3")
            h16_sb = cpool.tile([P, NJ * BL], F16, tag="h16")

            # 3D views [p, kc, *] for chunked DMA + DoubleRow k-pair slices
            xs8 = x8_sb[:].rearrange("p (kc b) -> p kc b", kc=KC)
            xd8 = x8.rearrange("p (kc b) -> p kc b", kc=KC)
            hs8 = h8_sb[:].rearrange("p (kc b) -> p kc b", kc=KC)
            hd8 = h8.rearrange("p (kc b) -> p kc b", kc=KC)
            xs3 = xe3_sb[:].rearrange("p (t b) -> p t b", t=NE3)
            xd3 = xe3.rearrange("p (t b) -> p t b", t=NE3)

            def gate_pairs(psum, w3, goff, b0, qs, start, stop, w=512):
                # pairs 0-3 read x8, pairs 4-7 read h8 (16 k-tiles over [x;h])
                for i, q in enumerate(qs):
                    src = xs8 if q < 4 else hs8
                    kk = (q % 4) * 2
                    g = goff + 2 * q
                    nc.tensor.matmul(
                        psum,
                        lhsT=w3[:, g : g + 2, :],
                        rhs=src[:, kk : kk + 2, b0 : b0 + w],
                        start=(start and i == 0),
                        stop=(stop and i == len(qs) - 1),
                        perf_mode=DR,
                    )

            def hc_dr(psum, w83, b0, w=512):
                for q in range(4):
                    nc.tensor.matmul(
                        psum,
                        lhsT=w83[:, 2 * q : 2 * q + 2, :],
                        rhs=hs8[:, 2 * q : 2 * q + 2, b0 : b0 + w],
                        start=(q == 0),
                        stop=(q == 3),
                        perf_mode=DR,
                    )

            def xc_mm(psum, w83, wc16_w, b0, w=512):
                # k-pairs 0..XCD-1: fp8e4 DR off resident x8;
                # k-tiles 2*XCD..7: full-rate e3m4 moving x, fp16 stationary.
                for q in range(XCD):
                    nc.tensor.matmul(
                        psum,
                        lhsT=w83[:, 8 + 2 * q : 8 + 2 * q + 2, :],
                        rhs=xs8[:, 2 * q : 2 * q + 2, b0 : b0 + w],
                        start=(q == 0),
                        stop=False,
                        perf_mode=DR,
                    )
                for t in range(NE3):
                    nc.tensor.matmul(
                        psum,
                        lhsT=wc16_w[:, t * P : (t + 1) * P],
                        rhs=xs3[:, t, b0 : b0 + w],
                        start=False,
                        stop=(t == NE3 - 1),
                    )

            def alloc_w():
                wg_w = wgpool.tile([P, 4096], F8, tag="wg")
                w8_w = wgpool.tile([P, 1792], F8, tag="w8")
                wc16_w = wgpool.tile([P, NE3 * P], F16, tag="wc16")
                return (
                    wg_w, w8_w, wc16_w,
                    wg_w[:].rearrange("p (g m) -> p g m", g=32),
                    w8_w[:].rearrange("p (g m) -> p g m", g=14),
                )

            def load_w(tiles, j):
                wg_w, w8_w, wc16_w = tiles[:3]
                nc.gpsimd.dma_start(wg_w[:], Wg[:, j * 4096 : (j + 1) * 4096])
                nc.gpsimd.dma_start(w8_w[:], W8[:, j * 1792 : (j + 1) * 1792])
                nc.gpsimd.dma_start(
                    wc16_w[:], Wc16[:, j * NE3 * P : (j + 1) * NE3 * P]
                )
                nc.gpsimd.dma_start(
                    h16_sb[:, j * BL : (j + 1) * BL], h16[:, j * BL : (j + 1) * BL]
                )

            def sig(psum, j, goff):
                g_sb = gpool.tile([P, 512], F16, tag="g")
                nc.scalar.activation(
                    g_sb[:], psum[:], AF.Sigmoid,
                    bias=bg_sb[:, goff + j : goff + j + 1], scale=inv,
                )
                return g_sb

            def blend(j, hoff, ph, px, r_sb, z_sb, w=512):
                # cand = tanh((px + r*(ph + bhc))/WS + bc)
                # out  = h + z*(cand - h)
                hsl = h16_sb[:, hoff : hoff + w]
                rh = wpool.tile([P, 512], F16, tag="rh")
                nc.vector.scalar_tensor_tensor(
                    rh[:, :w], ph, bhc_sb[:, j : j + 1], r_sb, ALU.add, ALU.mult
                )
                s = wpool.tile([P, 512], F32, tag="s")
                nc.vector.tensor_add(s[:, :w], px, rh[:, :w])
                cand = wpool.tile([P, 512], F16, tag="cand")
                nc.scalar.activation(
                    cand[:, :w], s[:, :w], AF.Tanh,
                    bias=bc_sb[:, j : j + 1], scale=inv,
                )
                d = wpool.tile([P, 512], F16, tag="d")
                nc.vector.tensor_sub(d[:, :w], cand[:, :w], hsl)
                m = wpool.tile([P, 512], F16, tag="m")
                nc.vector.tensor_mul(m[:, :w], z_sb, d[:, :w])
                o_sb = wpool.tile([P, 512], F16, tag="o")
                nc.vector.tensor_add(o_sb[:, :w], m[:, :w], hsl)
                nc.sync.dma_start(outT[:, hoff : hoff + w], o_sb[:, :w])

            # ---------------- j = 0: cold start ----------------
            cur = alloc_w()
            nxt = alloc_w()
            wg0, w80, wc160, wg03, w803 = cur

            # Pool queue: everything in consumption order. First the z-gate
            # weights (small first chunk so LDWEIGHTS starts early), then the
            # activation stream chunks interleaved with the remaining weights.
            nc.gpsimd.dma_start(wg0[:, 0:256], Wg[:, 0:256])
            nc.gpsimd.dma_start(xs8[:, 0:4, 0:512], xd8[:, 0:4, 0:512])
            nc.gpsimd.dma_start(wg0[:, 256:2048], Wg[:, 256:2048])
            nc.gpsimd.dma_start(wg0[:, 2048:4096], Wg[:, 2048:4096])
            nc.gpsimd.dma_start(hs8[:, 0:4, 0:512], hd8[:, 0:4, 0:512])
            nc.gpsimd.dma_start(xs8[:, 4:8, 0:512], xd8[:, 4:8, 0:512])
            nc.gpsimd.dma_start(hs8[:, 4:8, 0:512], hd8[:, 4:8, 0:512])
            nc.gpsimd.dma_start(bg_sb[:], bg[:])
            nc.gpsimd.dma_start(w80[:], W8[:, 0:1792])
            nc.gpsimd.dma_start(wc160[:], Wc16[:, 0 : NE3 * P])
            nc.gpsimd.dma_start(xs3[:, :, 0:512], xd3[:, :, 0:512])
            nc.gpsimd.dma_start(bhc_sb[:], bhc[:])
            nc.gpsimd.dma_start(bc_sb[:], bc[:])
            nc.gpsimd.dma_start(h16_sb[:, 0:BL], h16[:, 0:BL])
            # second half of the batch activations, then j1 prefetch
            nc.gpsimd.dma_start(xs8[:, :, 512:1024], xd8[:, :, 512:1024])
            nc.gpsimd.dma_start(hs8[:, :, 512:1024], hd8[:, :, 512:1024])
            nc.gpsimd.dma_start(xs3[:, :, 512:1024], xd3[:, :, 512:1024])
            load_w(nxt, 1)

            # j0/b0 compute in DMA-arrival order: both gates' x-pairs first.
            pz0 = ppool.tile([P, 512], F32, tag="ps")
            pr0 = ppool.tile([P, 512], F32, tag="ps")
            gate_pairs(pz0[:], wg03, 0, 0, (0, 1), True, False)
            gate_pairs(pr0[:], wg03, 16, 0, (0, 1), True, False)
            gate_pairs(pz0[:], wg03, 0, 0, (4, 5), False, False)
            gate_pairs(pr0[:], wg03, 16, 0, (4, 5), False, False)
            gate_pairs(pz0[:], wg03, 0, 0, (2, 3), False, False)
            gate_pairs(pr0[:], wg03, 16, 0, (2, 3), False, False)
            gate_pairs(pz0[:], wg03, 0, 0, (6, 7), False, True)
            gate_pairs(pr0[:], wg03, 16, 0, (6, 7), False, True)
            z0 = sig(pz0, 0, 0)
            r0 = sig(pr0, 0, NJ)
            ph0 = ppool.tile([P, 512], F32, tag="ps")
            hc_dr(ph0[:], w803, 0)
            px0 = ppool.tile([P, 512], F32, tag="ps")
            xc_mm(px0[:], w803, wc160, 0)
            blend(0, 0, ph0[:], px0[:], r0[:], z0[:])

            # j0/b1
            pz1 = ppool.tile([P, 512], F32, tag="ps")
            gate_pairs(pz1[:], wg03, 0, 512, range(8), True, True)
            z1 = sig(pz1, 0, 0)
            pr1 = ppool.tile([P, 512], F32, tag="ps")
            gate_pairs(pr1[:], wg03, 16, 512, range(8), True, True)
            r1 = sig(pr1, 0, NJ)
            ph1 = ppool.tile([P, 512], F32, tag="ps")
            hc_dr(ph1[:], w803, 512)
            px1 = ppool.tile([P, 512], F32, tag="ps")
            xc_mm(px1[:], w803, wc160, 512)
            blend(0, 512, ph1[:], px1[:], r1[:], z1[:])

            cur = nxt

            # ---------------- j = 1..7: steady state ----------------
            for j in range(1, NJ):
                if j + 1 < NJ:
                    nxt = alloc_w()
                    load_w(nxt, j + 1)
                wg_w, w8_w, wc16_w, wg3, w83 = cur

                for b in range(NB):
                    b0 = b * 512
                    hoff = j * BL + b0
                    last = j == NJ - 1 and b == NB - 1

                    pz = ppool.tile([P, 512], F32, tag="ps")
                    gate_pairs(pz[:], wg3, 0, b0, range(8), True, True)
                    z_sb = sig(pz, j, 0)
                    pr = ppool.tile([P, 512], F32, tag="ps")
                    gate_pairs(pr[:], wg3, 16, b0, range(8), True, True)
                    r_sb = sig(pr, j, NJ)

                    if not last:
                        ph = ppool.tile([P, 512], F32, tag="ps")
                        hc_dr(ph[:], w83, b0)
                        px = ppool.tile([P, 512], F32, tag="ps")
                        xc_mm(px[:], w83, wc16_w, b0)
                        blend(j, hoff, ph[:], px[:], r_sb[:], z_sb[:])
                    else:
                        # tail: 2x256 halves so the post-matmul chain is short
                        ph = ppool.tile([P, 512], F32, tag="ps")
                        px = ppool.tile([P, 512], F32, tag="ps")
                        for hv in range(2):
                            o0 = hv * 256
                            sl = slice(o0, o0 + 256)
                            hc_dr(ph[:, sl], w83, b0 + o0, w=256)
                            xc_mm(px[:, sl], w83, wc16_w, b0 + o0, w=256)
                            blend(
                                j, hoff + o0, ph[:, sl], px[:, sl],
                                r_sb[:, sl], z_sb[:, sl], w=256,
                            )

                cur = nxt

    nc.compile()
    return nc


def _pack_weights(W_ih, b_ih, W_hh, b_hh, W_c, b_c, W_hc, b_hc):
    f8 = ml_dtypes.float8_e4m3
    Wg_full = np.concatenate([W_ih, W_hh], axis=0)  # [2H, 2H] = [k, o]
    T = (Wg_full * WS).reshape(16, P, 16, P).transpose(1, 2, 0, 3)  # [p,ot,kt,o]
    WgH = np.ascontiguousarray(
        np.stack(
            [np.concatenate([T[:, j], T[:, NJ + j]], axis=1) for j in range(NJ)],
            axis=1,
        ).reshape(P, NJ * 4096)
    ).astype(f8)
    Thc = (W_hc * WS).reshape(KC, P, NJ, P).transpose(1, 2, 0, 3)  # [p,j,kt,o]
    Tc8 = (W_c[: 2 * XCD * P] * WS).reshape(2 * XCD, P, NJ, P).transpose(1, 2, 0, 3)
    W8H = np.ascontiguousarray(
        np.concatenate([Thc, Tc8], axis=2).reshape(P, NJ * 1792)
    ).astype(f8)
    Wc16H = np.ascontiguousarray(
        (W_c[2 * XCD * P :] * WS)
        .reshape(NE3, P, NJ, P)
        .transpose(1, 2, 0, 3)
        .reshape(P, NJ * NE3 * P)
    ).astype(np.float16)
    bgH = np.ascontiguousarray((b_ih + b_hh).reshape(16, P).T).astype(np.float32)
    bcH = np.ascontiguousarray(b_c.reshape(NJ, P).T).astype(np.float32)
    bhcH = np.ascontiguousarray((b_hc * WS).reshape(NJ, P).T).astype(np.float32)
    return WgH, W8H, Wc16H, bgH, bcH, bhcH


def _pack_acts(a, dtype, k0=0, k1=KC):
    # [BL, H] -> [p, kc*BL + b] with a[b, kc*128+p], k-tiles k0..k1-1
    return np.ascontiguousarray(
        a.T.reshape(KC, P, BL)[k0:k1].transpose(1, 0, 2).reshape(P, (k1 - k0) * BL)
    ).astype(dtype)


def _make_in_maps(input, hx, W_ih, b_ih, W_hh, b_hh, W_c, b_c, W_hc, b_hc):
    input = np.asarray(input, np.float32)
    hx = np.asarray(hx, np.float32)
    WgH, W8H, Wc16H, bgH, bcH, bhcH = _pack_weights(
        np.asarray(W_ih, np.float32), np.asarray(b_ih, np.float32),
        np.asarray(W_hh, np.float32), np.asarray(b_hh, np.float32),
        np.asarray(W_c, np.float32), np.asarray(b_c, np.float32),
        np.asarray(W_hc, np.float32), np.asarray(b_hc, np.float32),
    )
    f8 = ml_dtypes.float8_e4m3
    e3 = ml_dtypes.float8_e3m4
    in_maps = []
    for i in range(N_CORES):
        xs = input[i * BL : (i + 1) * BL]
        hs = hx[i * BL : (i + 1) * BL]
        in_maps.append(
            {
                "x8": _pack_acts(xs, f8),
                "h8": _pack_acts(hs, f8),
                "xe3": _pack_acts(xs, e3, k0=2 * XCD),
                "h16": _pack_acts(hs, np.float16),
                "Wg": WgH,
                "W8": W8H,
                "Wc16": Wc16H,
                "bg": bgH,
                "bc": bcH,
                "bhc": bhcH,
            }
        )
    return in_maps


def kernel(input, hx, W_ih, b_ih, W_hh, b_hh, W_c, b_c, W_hc, b_hc):
    if "nc" not in _CACHE:
        _CACHE["nc"] = _build_program()
    nc = _CACHE["nc"]

    in_maps = _make_in_maps(
        input, hx, W_ih, b_ih, W_hh, b_hh, W_c, b_c, W_hc, b_hc
    )

    res = run_bass_kernel_spmd(nc, in_maps, core_ids=list(range(N_CORES)))
    out = np.empty((B, H), np.float32)
    for i, r in enumerate(res.results):
        o = np.asarray(r["outT"], np.float32).reshape(P, NJ, BL).transpose(2, 1, 0).reshape(BL, H)
        out[i * BL : (i + 1) * BL] = o
    return out
